# revision 1
# baseline (speedup 1.0000x reference)
"""Causal self-attention Trainium2 Bass kernel.

Problem: B=16, T=2048, D=128, H=4 (head dim 32), fp32 in/out.
  qkv = x @ w_attn ; per-head scores = q k^T / sqrt(32), causal
  y = softmax(scores) @ v ; out = y @ w_proj

Sharding: data-parallel over batch, 2 batches per NeuronCore x 8 cores
(SPMD, no collectives).

Per-core pipeline (measured 412 us/core on TRN2 vs 810 us for the pure
fp32 variant; flip CONFIG["attn_dt"] to "f32" for bit-conservative mode):

  phase A: x tiles DMA'd in 512-token blocks, PE-transposed -> xT [128d, T]
    (fp32); QKV^T projection (fp32): Q^T/K^T stay head-major [128ch, T]
    (head h lives on partitions 32h:32h+32 = exactly the K=32 contraction
    layout the scores matmuls need); V is produced token-major and stored
    per head as V' [128k, 128] = [V_h | ones | zeros] in fp16 - the ones
    column makes the attn@V matmul emit the softmax denominator row for
    free (row 32 for even heads, 64 for odd heads).

  phase B, per (batch, 512-wide q-group, head-pair), fp16 operands:
    for each causal k-chunk (128 wide):
      scores^T [128k, 2h, 512q] = two K=32 matmuls packed into the PE
        array with tile_position=(32h, 0) row tiling
      diagonal-band chunks: -1e9 additive mask on the 128x128 diagonal
        block (DVE), exp skips the fully-masked region left of it
      exp on ScalarE straight out of the 2-bank PSUM quad (scale fused,
        fp16 out), double-buffered quads keep ACT saturated
      attn@V: per head one M=128 matmul (fp16) accumulating y^T + sums
        into that head's PSUM bank
    normalize: sums rows staged into a zeroed SBUF tile (partition-aligned
      copies), one expander matmul broadcasts both heads' sums to
      partitions 0:64, DVE reciprocal + two multiplies -> ysbP (even head
      rows 0:32, odd rows 32:64, rest kept zero)
    first 128 query rows are recomputed exactly in fp32 (they attend over
      few keys, so fp16 rounding is not averaged away) and overwrite the
      fp16 result in y columns 0:128 before normalization
    out projection: two accumulating K=128 fp32 matmuls per 128-token
      chunk against per-pair row-packed w_proj tiles; 512-token batched
      DMA out.

Hardware traps encountered (avoid on TRN2):
  - M<128 col-tiled matmuls (tile_position=(0,64), M=33) crash the device
    for fp32 and fp16 accumulation breaks silently at M=64; all attn@V
    matmuls here use M=128.
  - Accumulating K=32 matmuls at *different* row positions into one PSUM
    bank crashes; same-position accumulation is fine.
  - float32r needs DVE-produced operands (special storage format), and is
    invalid ISA for col-offset tile positions; fp16 is the usable fast
    dtype (ACT can write it directly).
  - fp32 matmul costs 4 cycles/row (two half-speed passes); fp16 is 1.
  - Engine partition offsets must be 32-aligned; cross-operand partition
    shifts in 32-steps are legal on DVE.
"""

import os
import sys

import numpy as np

for _p in ("/opt/trn_rl_repo", "/root/.axon_site/_ro/trn_rl_repo"):
    if os.path.isdir(_p) and _p not in sys.path:
        sys.path.insert(0, _p)

import concourse.bass as bass
import concourse.bacc as bacc
import concourse.mybir as mybir
import concourse.tile as tile
from concourse.bass_utils import run_bass_kernel_spmd

F32 = mybir.dt.float32
P = 128
NEG = -1.0e9

# full problem shape (hardcoded per harness contract)
B, T, D, H = 16, 2048, 128, 4
DH = D // H  # 32
N_CORES = 8
BPC = B // N_CORES  # batches per core


def build_attention_nc(bpc=BPC, t=T, qg=512, loop_n=0, attn_dt="f32",
                       quad_bufs=1, exp_bufs=3, av_f32=False):
    """Build the single-core SPMD Bass program.

    bpc: batches this core handles; t: sequence length; qg: q-group width.
    """
    assert t % qg == 0 and qg % P == 0 and t % P == 0
    nqg = t // qg      # q groups
    nkc = t // P       # 128-wide k chunks
    cpq = qg // P      # k chunks per q group
    scale = 1.0 / float(np.sqrt(DH))
    # attention matmul operand dtype: fp16 runs the PE at 1 cycle/row
    # (vs 4 for fp32); softmax statistics strongly attenuate the rounding
    FA = {"f32": F32, "f16": mybir.dt.float16,
          "f32r": mybir.dt.float32r}[attn_dt]
    FV = F32 if av_f32 else FA  # attn@V operand dtype (exp tiles + V')

    nc = bacc.Bacc("TRN2", target_bir_lowering=False, debug=False)
    x_d = nc.dram_tensor("x", [bpc, t, D], F32, kind="ExternalInput")
    wa_d = nc.dram_tensor("w_attn", [D, 3 * D], F32, kind="ExternalInput")
    wp_d = nc.dram_tensor("w_proj", [D, D], F32, kind="ExternalInput")
    out_d = nc.dram_tensor("out", [bpc, t, D], F32, kind="ExternalOutput")

    with tile.TileContext(nc) as tc:
        with tc.tile_pool(name="resident", bufs=1) as res:
            # ---- constants ----
            wa_sb = res.tile([D, 3 * D], F32, name="wa", tag="wa")
            nc.sync.dma_start(wa_sb[:], wa_d[:])
            # w_proj rows permuted to match where heads land in the y psum
            # tiles: pass A holds heads {0,1} at partitions {0:32, 64:96},
            # pass B heads {2,3} likewise.
            # per-pair w_proj tiles: rows 0:32 = even head's w_proj rows,
            # rows 32:64 = odd head's, rest zero (the out projection
            # contracts all 128 partitions; ysb rows 64:128 likewise zero)
            wpP = [res.tile([D, D], F32, name=f"wp{pi}", tag=f"wp{pi}")
                   for pi in range(2)]
            for pi in range(2):
                nc.vector.memset(wpP[pi][:], 0.0)
                for ci in range(2):
                    h = 2 * pi + ci
                    nc.sync.dma_start(wpP[pi][DH * ci:DH * (ci + 1), :],
                                      wp_d[h * DH:(h + 1) * DH, :])

            ident = res.tile([P, P], F32, name="ident", tag="ident")
            nc.gpsimd.memset(ident[:], 0.0)
            nc.gpsimd.affine_select(
                out=ident[:], in_=ident[:],
                compare_op=mybir.AluOpType.not_equal, fill=1.0,
                base=0, pattern=[[-1, P]], channel_multiplier=1,
            )

            # causal additive mask for the diagonal 128x128 block:
            # cmask[k, c] = 0 if c >= k else NEG
            cmask = res.tile([P, P], F32, name="cmask", tag="cmask")
            nc.gpsimd.memset(cmask[:], 0.0)
            nc.gpsimd.affine_select(
                out=cmask[:], in_=cmask[:],
                compare_op=mybir.AluOpType.is_ge, fill=NEG,
                base=0, pattern=[[1, P]], channel_multiplier=-1,
            )

            # combined expander: maps the pair's sums rows (32 -> partitions
            # 0:32 for the even head, 64 -> 32:64 for the odd head)
            exp_e = res.tile([P, 2 * DH], F32, name="exp_e", tag="exp_e")
            nc.gpsimd.memset(exp_e[:], 0.0)
            nc.gpsimd.memset(exp_e[32:33, 0:DH], 1.0)
            nc.gpsimd.memset(exp_e[64:65, DH:2 * DH], 1.0)

            # sums staging tiles (zeros except rows 32 / 96, rewritten per
            # q-group; zero rows make the expander matmul contraction clean)
            s_p = [res.tile([P, qg], F32, name=f"s_p{pi}", tag=f"s_p{pi}")
                   for pi in range(2)]
            for pi in range(2):
                nc.vector.memset(s_p[pi][:], 0.0)
            # resident normalized-y tiles, one per pair (even head rows 0:32,
            # odd head rows 32:64); rows 64:128 stay zero so the out
            # projection can contract all 128 partitions
            ysbP = [res.tile([P, qg], F32, name=f"ysbp{pi}", tag=f"ysbp{pi}")
                    for pi in range(2)]
            for pi in range(2):
                nc.vector.memset(ysbP[pi][:], 0.0)

            # fp32 shadows of q/k (first 128 tokens) and V chunk 0: the
            # first 128 query rows attend over few keys, so fp16 rounding
            # is not averaged away there; they are recomputed exactly and
            # overwrite the fp16 result
            qT32 = [res.tile([P, P], F32, name=f"qT32_{b}", tag=f"qT32_{b}")
                    for b in range(bpc)]
            kT32 = [res.tile([P, P], F32, name=f"kT32_{b}", tag=f"kT32_{b}")
                    for b in range(bpc)]
            vP32 = [res.tile([P, H, P], F32, name=f"vP32_{b}", tag=f"vP32_{b}")
                    for b in range(bpc)]
            for b in range(bpc):
                nc.gpsimd.memset(vP32[b][:], 0.0)
                nc.gpsimd.memset(vP32[b][:, 0::2, 32:33], 1.0)
                nc.gpsimd.memset(vP32[b][:, 1::2, 64:65], 1.0)

            # ---- per-batch resident activations ----
            xT = [res.tile([P, t], F32, name=f"xT{b}", tag=f"xT{b}") for b in range(bpc)]
            qT = [res.tile([P, t], FA, name=f"qT{b}", tag=f"qT{b}") for b in range(bpc)]
            kT = [res.tile([P, t], FA, name=f"kT{b}", tag=f"kT{b}") for b in range(bpc)]
            # V' per (b, h): [128 kpos, nkc, 64] = [V_h | ones | zeros].
            # 64 wide because M=33 col-tiled matmuls crash the device
            # (NRT_EXEC_UNIT_UNRECOVERABLE); M=64 at positions (0,0)/(0,64)
            # is solid. Col 32 stays 1.0 (softmax denominator trick).
            vP = [res.tile([P, nkc, H, P], FV, name=f"vp{b}", tag=f"vp{b}")
                  for b in range(bpc)]
            for b in range(bpc):
                nc.gpsimd.memset(vP[b][:], 0.0)
                # ones column at 32 for even heads, 64 for odd heads (the
                # pair's two sums land on different psum rows so both can be
                # staged into one s tile with partition-aligned copies)
                nc.gpsimd.memset(vP[b][:, :, 0::2, 32:33], 1.0)
                nc.gpsimd.memset(vP[b][:, :, 1::2, 64:65], 1.0)

            # optional hardware repeat loop (timing measurements only)
            import contextlib
            loop_cm = (tc.For_i(0, loop_n, 1) if loop_n
                       else contextlib.nullcontext())
            with loop_cm:
              # ============== phase A: x^T and QKV^T ==============
              with (
                  tc.tile_pool(name="xin", bufs=8) as xin_pool,
                  tc.tile_pool(name="psA", bufs=2, space="PSUM") as psA,
                  tc.tile_pool(name="psQK", bufs=2, space="PSUM") as psQK,
              ):
                  for b in range(bpc):
                      for kc in range(nkc):
                          xi = xin_pool.tile([P, D], F32, name="xin", tag="xin")
                          nc.sync.dma_start(xi[:], x_d[b, kc * P:(kc + 1) * P, :])
                          pst = psA.tile([P, P], F32, name="pst", tag="pst")
                          nc.tensor.transpose(pst[:], xi[:], ident[:])
                          nc.vector.tensor_copy(
                              xT[b][:, kc * P:(kc + 1) * P], pst[:])
                      # Q^T / K^T head-major
                      for j in range(t // 512):
                          sl = slice(j * 512, (j + 1) * 512)
                          for wofs, dst in ((0, qT[b]), (D, kT[b])):
                              pq = psQK.tile([P, 512], F32, name="pq", tag="pq")
                              nc.tensor.matmul(
                                  pq[:], wa_sb[:, wofs:wofs + D], xT[b][:, sl],
                                  start=True, stop=True)
                              nc.vector.tensor_copy(dst[:, sl], pq[:])
                              if j == 0:
                                  dst32 = qT32[b] if wofs == 0 else kT32[b]
                                  nc.vector.tensor_copy(dst32[:],
                                                        pq[:, 0:P])
                      # V token-major, scattered into per-head V' tiles
                      for kc in range(nkc):
                          pv = psA.tile([P, P], F32, name="pv", tag="pv")
                          nc.tensor.matmul(
                              pv[:], xT[b][:, kc * P:(kc + 1) * P],
                              wa_sb[:, 2 * D:3 * D], start=True, stop=True)
                          nc.vector.tensor_copy(
                              vP[b][:, kc, :, 0:DH],
                              pv[:].rearrange("p (h d) -> p h d", h=H))
                          if kc == 0:
                              nc.vector.tensor_copy(
                                  vP32[b][:, :, 0:DH],
                                  pv[:].rearrange("p (h d) -> p h d", h=H))

              # ================= phase B: attention =================
              with (
                  tc.tile_pool(name="quad", bufs=quad_bufs, space="PSUM") as quad_pool,
                  tc.tile_pool(name="ypsum", bufs=2, space="PSUM") as y_pool,
                  tc.tile_pool(name="aux", bufs=1, space="PSUM") as aux_pool,
                  tc.tile_pool(name="expt", bufs=exp_bufs) as exp_pool,
                  tc.tile_pool(name="yT", bufs=2) as yt_pool,
                  tc.tile_pool(name="outsb", bufs=6) as out_pool,
                  tc.tile_pool(name="rsb", bufs=4) as r_pool,
              ):
                  for b in range(bpc):
                      for j in range(nqg):
                          qsl = slice(j * qg, (j + 1) * qg)
                          kmax = cpq * (j + 1) - 1
                          for pi in range(2):  # head pairs (0,1), (2,3)
                              y_p = [y_pool.tile([P, qg], F32, name="y",
                                                 tag="y") for _ in range(2)]
                              for kc in range(kmax + 1):
                                  ksl = slice(kc * P, (kc + 1) * P)
                                  quad = quad_pool.tile([P, 2, qg], F32,
                                                        name="quad", tag="quad")
                                  for ci in range(2):
                                      h = 2 * pi + ci
                                      hp = slice(32 * h, 32 * h + 32)
                                      nc.tensor.matmul(
                                          quad[:, ci, :], kT[b][hp, ksl],
                                          qT[b][hp, qsl],
                                          start=True, stop=True,
                                          tile_position=(32 * h, 0))
                                  r = kc - cpq * j  # diag band index
                                  et = exp_pool.tile([P, 2, qg], FV,
                                                     name="et", tag="et")
                                  if r >= 0:
                                      blk = slice(r * P, (r + 1) * P)
                                      nc.vector.tensor_tensor(
                                          quad[:, :, blk], quad[:, :, blk],
                                          cmask[:, None, :].to_broadcast(
                                              (P, 2, P)),
                                          mybir.AluOpType.add)
                                      if r > 0:
                                          nc.gpsimd.memset(
                                              et[:, :, 0:r * P], 0.0)
                                      nc.scalar.activation(
                                          et[:, :, r * P:],
                                          quad[:, :, r * P:],
                                          mybir.ActivationFunctionType.Exp,
                                          scale=scale)
                                  else:
                                      nc.scalar.activation(
                                          et[:], quad[:],
                                          mybir.ActivationFunctionType.Exp,
                                          scale=scale)
                                  st = kc == 0
                                  sp = kc == kmax
                                  for ci in range(2):
                                      h = 2 * pi + ci
                                      nc.tensor.matmul(
                                          y_p[ci][:], vP[b][:, kc, h, :],
                                          et[:, ci, :],
                                          start=st, stop=sp,
                                          skip_group_check=True)
                              if j == 0:
                                  # exact fp32 recompute of query rows 0:128
                                  # (kc=0 only); overwrites the fp16 result
                                  # in y columns 0:128
                                  q32 = quad_pool.tile([P, 2, qg], F32,
                                                       name="q32", tag="quad")
                                  for ci in range(2):
                                      h = 2 * pi + ci
                                      hp = slice(32 * h, 32 * h + 32)
                                      nc.tensor.matmul(
                                          q32[:, ci, 0:P], kT32[b][hp, :],
                                          qT32[b][hp, :],
                                          start=True, stop=True,
                                          tile_position=(32 * h, 0))
                                  nc.vector.tensor_tensor(
                                      q32[:, :, 0:P], q32[:, :, 0:P],
                                      cmask[:, None, :].to_broadcast(
                                          (P, 2, P)),
                                      mybir.AluOpType.add)
                                  et32 = r_pool.tile([P, 2, P], F32,
                                                     name="et32", tag="et32")
                                  nc.scalar.activation(
                                      et32[:], q32[:, :, 0:P],
                                      mybir.ActivationFunctionType.Exp,
                                      scale=scale)
                                  for ci in range(2):
                                      h = 2 * pi + ci
                                      nc.tensor.matmul(
                                          y_p[ci][:, 0:P],
                                          vP32[b][:, h, :], et32[:, ci, :],
                                          start=True, stop=True,
                                          skip_group_check=True)
                              # ---- normalize this pair ----
                              s_t = s_p[pi]
                              nc.vector.tensor_copy(s_t[32:33, :],
                                                    y_p[0][32:33, :])
                              nc.vector.tensor_copy(s_t[64:65, :],
                                                    y_p[1][64:65, :])
                              ps_r = aux_pool.tile([P, qg], F32,
                                                   name="psr", tag="psr")
                              nc.tensor.matmul(ps_r[0:2 * DH, :],
                                               exp_e[:], s_t[:],
                                               start=True, stop=True)
                              rec = r_pool.tile([2 * DH, qg], F32,
                                                name="rec", tag="rec")
                              nc.vector.reciprocal(rec[:], ps_r[0:2 * DH, :])
                              nc.vector.tensor_mul(
                                  ysbP[pi][0:DH, :], y_p[0][0:DH, :],
                                  rec[0:DH, :])
                              nc.vector.tensor_mul(
                                  ysbP[pi][DH:2 * DH, :], y_p[1][0:DH, :],
                                  rec[DH:2 * DH, :])
                          # ---- output projection ----
                          for tch in range(qg // P):
                              t0 = j * qg + tch * P
                              csl = slice(tch * P, (tch + 1) * P)
                              po = aux_pool.tile([P, D], F32,
                                                 name="proj", tag="proj")
                              for pi in range(2):
                                  nc.tensor.matmul(
                                      po[:], ysbP[pi][:, csl], wpP[pi][:],
                                      start=(pi == 0), stop=(pi == 1),
                                      skip_group_check=True)
                              ob = out_pool.tile([P, D], F32,
                                                 name="ob", tag="ob")
                              nc.vector.tensor_copy(ob[:], po[:])
                              nc.sync.dma_start(out_d[b, t0:t0 + P, :], ob[:])
    nc.compile()
    return nc


_NC_CACHE = {}

# shipped configuration: fp16 attention operands (PE at 1 cycle/row vs 4
# for fp32) with an exact-fp32 recompute of the first 128 query rows;
# measured absmax vs fp64 reference ~3.5e-5 (fp32-exact fallback:
# attn_dt="f32").
CONFIG = {"attn_dt": "f16", "quad_bufs": 2, "exp_bufs": 6}


def _get_nc(bpc=BPC, t=T, loop_n=0):
    key = (bpc, t, loop_n)
    if key not in _NC_CACHE:
        _NC_CACHE[key] = build_attention_nc(bpc=bpc, t=t, loop_n=loop_n,
                                            **CONFIG)
    return _NC_CACHE[key]


def _run(x, w_attn, w_proj, **spmd_kwargs):
    x = np.ascontiguousarray(np.asarray(x), dtype=np.float32)
    w_attn = np.ascontiguousarray(np.asarray(w_attn), dtype=np.float32)
    w_proj = np.ascontiguousarray(np.asarray(w_proj), dtype=np.float32)
    assert x.shape == (B, T, D), x.shape

    nc = _get_nc()
    in_maps = [
        {"x": x[c * BPC:(c + 1) * BPC], "w_attn": w_attn, "w_proj": w_proj}
        for c in range(N_CORES)
    ]
    res = run_bass_kernel_spmd(nc, in_maps, list(range(N_CORES)),
                               **spmd_kwargs)
    out = np.concatenate([res.results[c]["out"] for c in range(N_CORES)],
                         axis=0)
    return out.astype(np.float32), res


def kernel(x, w_attn, w_proj):
    out, _ = _run(x, w_attn, w_proj)
    return out


if __name__ == "__main__":
    nc = build_attention_nc()
    print("built ok")



# revision 2
# speedup vs baseline: 1.1565x; 1.1565x over previous
"""Causal self-attention Trainium2 Bass kernel (v2).

Problem: B=16, T=2048, D=128, H=4 (head dim 32), fp32 in/out.
  qkv = x @ w_attn ; per-head scores = q k^T / sqrt(32), causal
  y = softmax(scores) @ v ; out = y @ w_proj

Sharding: data-parallel over batch, 2 batches per NeuronCore x 8 cores
(SPMD, no collectives).

v2 design (vs the 377us v1 baseline):
  - fp16 operands for EVERY matmul (PE 1 cycle/row vs 4 for fp32):
    QKV projection, scores, attn@V, normalize expander, out projection.
  - exp is split across engines: ACT runs the real exp (activation
    table); a configurable fraction of score chunks run a one-op
    Schraudolph fast-exp on DVE instead ( int16(x*A+B) bitcast to fp16,
    ~1.8% rms method error, saturates to -0.0 = exact zero for masked
    -1e9 inputs ). Pool (gpsimd) cannot read PSUM so it zero-fills the
    causal triangle with affine_select on the fp16 et tiles post-exp,
    replacing the fp32 additive -1e9 mask entirely.
  - scores / attn@V matmuls are narrowed to the causally-valid q range
    of each 128-wide k chunk (moving rows = cost on PE).
  - normalize + out-projection of each (group, pair) are deferred and
    re-emitted piecewise inside the NEXT group's chunk loop so the PE
    never idles on the DVE normalize chain; phase A of batch b+1 is
    likewise drained piecewise through phase B of batch b.
  - single shared PSUM pool layout: quad pool 2x2 banks + y pool 4x1
    banks = 8 banks exactly; expander/out-proj psum tiles borrow quad
    pool slots.

Numerics: rel err vs fp32 reference ~1e-2 (gate 2e-2). Error budget is
dominated by the Schraudolph share (sqrt(phi)*1.8e-2); phi=0.25 here.
fp8 variants measured ~2.1e-2 end to end (attention output is itself an
average, so weight noise does not average out) and were rejected.
"""

import os
import sys
from collections import deque

import numpy as np

for _p in ("/opt/trn_rl_repo", "/root/.axon_site/_ro/trn_rl_repo"):
    if os.path.isdir(_p) and _p not in sys.path:
        sys.path.insert(0, _p)

import concourse.bass as bass
import concourse.bacc as bacc
import concourse.mybir as mybir
import concourse.tile as tile
from concourse.bass_utils import run_bass_kernel_spmd

F32 = mybir.dt.float32
F16 = mybir.dt.float16
I16 = mybir.dt.int16
P = 128

# full problem shape (hardcoded per harness contract)
B, T, D, H = 16, 2048, 128, 4
DH = D // H  # 32
N_CORES = 8
BPC = B // N_CORES  # batches per core

SCALE = 1.0 / float(np.sqrt(DH))
# Schraudolph fast-exp: exp(s*SCALE) ~ bitcast_f16(int16(s*A16 + B16))
A16 = 1024.0 * 1.4426950408889634 * SCALE
B16 = 15315.25  # RNE-calibrated offset (max rel err ~3.0%)


def build_attention_nc(bpc=BPC, t=T, qg=512, loop_n=0,
                       exp_pat="ADAA", tri_engine="pool",
                       quad_bufs=2, y_bufs=4, exp_bufs=4,
                       overlap_a=True):
    """Build the single-core SPMD Bass program."""
    assert t % qg == 0 and qg % P == 0
    nqg = t // qg          # q groups
    nkc = t // P           # 128-wide k chunks
    cpq = qg // P          # k chunks per q group (4)

    nc = bacc.Bacc("TRN2", target_bir_lowering=False, debug=False)
    x_d = nc.dram_tensor("x", [bpc, t, D], F32, kind="ExternalInput")
    wa_d = nc.dram_tensor("w_attn", [D, 3 * D], F32, kind="ExternalInput")
    wp_d = nc.dram_tensor("w_proj", [D, D], F32, kind="ExternalInput")
    out_d = nc.dram_tensor("out", [bpc, t, D], F32, kind="ExternalOutput")

    with tile.TileContext(nc) as tc:
        with tc.tile_pool(name="resident", bufs=1) as res:
            # ---- weights ----
            wa32 = res.tile([D, 3 * D], F32, name="wa32", tag="wa32")
            nc.sync.dma_start(wa32[:], wa_d[:])
            wa16 = res.tile([D, 3 * D], F16, name="wa16", tag="wa16")
            nc.vector.tensor_copy(wa16[:], wa32[:])
            wp32 = res.tile([D, D], F32, name="wp32", tag="wp32")
            nc.sync.dma_start(wp32[:], wp_d[:])
            # per-pair w_proj tiles: rows 0:32 = even head's w_proj rows,
            # rows 32:64 = odd head's, rest zero (out projection contracts
            # all 128 partitions; ysbP rows 64:128 likewise zero)
            wp16P = [res.tile([D, D], F16, name=f"wp{pi}", tag=f"wp{pi}")
                     for pi in range(2)]
            for pi in range(2):
                nc.vector.memset(wp16P[pi][:], 0.0)
                for ci in range(2):
                    h = 2 * pi + ci
                    nc.vector.tensor_copy(
                        wp16P[pi][DH * ci:DH * (ci + 1), :],
                        wp32[h * DH:(h + 1) * DH, :])

            ident = res.tile([P, P], F32, name="ident", tag="ident")
            nc.gpsimd.memset(ident[:], 0.0)
            nc.gpsimd.affine_select(
                out=ident[:], in_=ident[:],
                compare_op=mybir.AluOpType.not_equal, fill=1.0,
                base=0, pattern=[[-1, P]], channel_multiplier=1,
            )

            # fp16 0/1 lower-triangle tile for the dve-mul tri fallback
            tri16 = res.tile([P, P], F16, name="tri16", tag="tri16")
            nc.gpsimd.memset(tri16[:], 1.0)
            nc.gpsimd.affine_select(
                out=tri16[:], in_=tri16[:],
                compare_op=mybir.AluOpType.is_ge, fill=0.0,
                base=0, pattern=[[1, P]], channel_multiplier=-1,
            )

            # expander: maps sums rows (32 -> partitions 0:32 even head,
            # 64 -> 32:64 odd head)
            exp_e = res.tile([P, 2 * DH], F16, name="exp_e", tag="exp_e")
            nc.gpsimd.memset(exp_e[:], 0.0)
            nc.gpsimd.memset(exp_e[32:33, 0:DH], 1.0)
            nc.gpsimd.memset(exp_e[64:65, DH:2 * DH], 1.0)

            # sums staging (zeros except rows 32/64, rewritten per group)
            s_p = [res.tile([P, qg], F16, name=f"s_p{pi}", tag=f"s_p{pi}")
                   for pi in range(2)]
            for pi in range(2):
                nc.vector.memset(s_p[pi][:], 0.0)
            # normalized-y staging, per pair (even head rows 0:32, odd
            # 32:64, rows 64:128 stay zero for the full-K out projection)
            ysbP = [res.tile([P, qg], F16, name=f"ysbp{pi}", tag=f"ysbp{pi}")
                    for pi in range(2)]
            for pi in range(2):
                nc.vector.memset(ysbP[pi][:], 0.0)

            # ---- per-batch resident activations (all fp16) ----
            xT = [res.tile([P, t], F16, name=f"xT{b}", tag=f"xT{b}")
                  for b in range(bpc)]
            qT = [res.tile([P, t], F16, name=f"qT{b}", tag=f"qT{b}")
                  for b in range(bpc)]
            kT = [res.tile([P, t], F16, name=f"kT{b}", tag=f"kT{b}")
                  for b in range(bpc)]
            # V' per (b, kc, h): [128 kpos, 128] = [V_h | ones | zeros];
            # ones col 32 (even head) / 64 (odd head) emits the softmax
            # denominator row for free in the attn@V matmul.
            vP = [res.tile([P, nkc, H, P], F16, name=f"vp{b}", tag=f"vp{b}")
                  for b in range(bpc)]
            for b in range(bpc):
                nc.gpsimd.memset(vP[b][:], 0.0)
                nc.gpsimd.memset(vP[b][:, :, 0::2, 32:33], 1.0)
                nc.gpsimd.memset(vP[b][:, :, 1::2, 64:65], 1.0)

            import contextlib
            loop_cm = (tc.For_i(0, loop_n, 1) if loop_n
                       else contextlib.nullcontext())
            with loop_cm:
              with (
                  tc.tile_pool(name="quad", bufs=quad_bufs,
                               space="PSUM") as quad_pool,
                  tc.tile_pool(name="ypsum", bufs=y_bufs,
                               space="PSUM") as y_pool,
                  tc.tile_pool(name="expt", bufs=exp_bufs) as expt,
                  tc.tile_pool(name="xin", bufs=8) as xin_pool,
                  tc.tile_pool(name="outsb", bufs=3) as out_pool,
                  tc.tile_pool(name="rsb", bufs=4) as r_pool,
              ):
                # two deferred-work queues re-emitted piecewise inside
                # later chunk loops: tails (normalize/proj; must drain
                # promptly to free y psum slots) and lowq (next batch's
                # phase A)
                tails = deque()
                lowq = deque()
                gctr = [0]

                def drain(n=1):
                    for _ in range(n):
                        if tails:
                            tails.popleft()()
                        elif lowq:
                            lowq.popleft()()
                        else:
                            break

                # ---------- phase A pieces for batch b ----------
                def phase_a_pieces(b):
                    pieces = []

                    def mk_x(kc):
                        def p():
                            xi = xin_pool.tile([P, D], F32, name="xin",
                                               tag="xin")
                            nc.sync.dma_start(
                                xi[:], x_d[b, kc * P:(kc + 1) * P, :])
                            pst = quad_pool.tile([P, P], F32, name="pst",
                                                 tag="quad")
                            nc.tensor.transpose(pst[:], xi[:], ident[:])
                            nc.scalar.copy(
                                xT[b][:, kc * P:(kc + 1) * P], pst[:])
                        return p

                    def mk_qk(j4, wofs):
                        def p():
                            sl = slice(j4 * 512, (j4 + 1) * 512)
                            pq = quad_pool.tile([P, 512], F32, name="pq",
                                                tag="quad")
                            nc.tensor.matmul(
                                pq[:], wa16[:, wofs:wofs + D], xT[b][:, sl],
                                start=True, stop=True)
                            dst = qT[b] if wofs == 0 else kT[b]
                            eng = nc.vector if wofs == 0 else nc.scalar
                            if wofs == 0:
                                eng.tensor_copy(dst[:, sl], pq[:])
                            else:
                                eng.copy(dst[:, sl], pq[:])
                        return p

                    def mk_v(kc):
                        def p():
                            pv = quad_pool.tile([P, P], F32, name="pv",
                                                tag="quad")
                            nc.tensor.matmul(
                                pv[:], xT[b][:, kc * P:(kc + 1) * P],
                                wa16[:, 2 * D:3 * D], start=True, stop=True)
                            nc.scalar.copy(
                                vP[b][:, kc, :, 0:DH],
                                pv[:].rearrange("p (h d) -> p h d", h=H))
                        return p

                    for kc in range(nkc):
                        pieces.append(mk_x(kc))
                    for j4 in range(t // 512):
                        pieces.append(mk_qk(j4, 0))
                        pieces.append(mk_qk(j4, D))
                    for kc in range(nkc):
                        pieces.append(mk_v(kc))
                    return pieces

                # ---------- phase B helpers ----------
                def emit_exp(et_ap, quad_ap):
                    c = exp_pat[gctr[0] % len(exp_pat)]
                    gctr[0] += 1
                    if c == "A":
                        nc.scalar.activation(
                            et_ap, quad_ap,
                            mybir.ActivationFunctionType.Exp, scale=SCALE)
                    else:
                        nc.vector.tensor_scalar(
                            out=et_ap.bitcast(I16), in0=quad_ap,
                            scalar1=A16, scalar2=B16,
                            op0=mybir.AluOpType.mult,
                            op1=mybir.AluOpType.add)

                def tri_mask(et, lo):
                    ap = et[:, :, lo:lo + P]
                    if tri_engine == "pool":
                        nc.gpsimd.affine_select(
                            out=ap, in_=ap,
                            compare_op=mybir.AluOpType.is_ge, fill=0.0,
                            base=0, pattern=[[0, 2], [1, P]],
                            channel_multiplier=-1)
                    else:
                        nc.vector.tensor_tensor(
                            ap, ap,
                            tri16[:, None, :].to_broadcast((P, 2, P)),
                            mybir.AluOpType.mult)

                def mk_c1(y_p, pi):
                    def c1():
                        nc.vector.tensor_copy(s_p[pi][32:33, :],
                                              y_p[0][32:33, :])
                        nc.scalar.copy(s_p[pi][64:65, :], y_p[1][64:65, :])
                    return c1

                def mk_c2(pi, rec_box):
                    def c2():
                        ps_r = quad_pool.tile([P, qg], F32, name="psr",
                                              tag="quad")
                        nc.tensor.matmul(ps_r[0:2 * DH, :], exp_e[:],
                                         s_p[pi][:], start=True, stop=True)
                        rec = r_pool.tile([2 * DH, qg], F32, name="rec",
                                          tag="rec")
                        nc.vector.reciprocal(rec[:], ps_r[0:2 * DH, :])
                        rec_box[0] = rec
                    return c2

                def mk_c3(y_p, pi, rec_box):
                    def c3():
                        rec = rec_box[0]
                        nc.vector.tensor_mul(
                            ysbP[pi][0:DH, :], y_p[0][0:DH, :], rec[0:DH, :])
                        nc.vector.tensor_mul(
                            ysbP[pi][DH:2 * DH, :], y_p[1][0:DH, :],
                            rec[DH:2 * DH, :])
                    return c3

                def mk_proj(b, j):
                    def pj():
                        po4 = quad_pool.tile([P, cpq, D], F32, name="po4",
                                             tag="quad")
                        for cc in range(cpq):
                            csl = slice(cc * P, (cc + 1) * P)
                            for pi2 in range(2):
                                nc.tensor.matmul(
                                    po4[:, cc, :], ysbP[pi2][:, csl],
                                    wp16P[pi2][:],
                                    start=(pi2 == 0), stop=(pi2 == 1),
                                    skip_group_check=True)
                        ob = out_pool.tile([P, cpq, D], F32, name="ob",
                                           tag="ob")
                        nc.scalar.copy(ob[:], po4[:])
                        for cc in range(cpq):
                            t0 = j * qg + cc * P
                            nc.sync.dma_start(out_d[b, t0:t0 + P, :],
                                              ob[:, cc, :])
                    return pj

                # ---------- main schedule ----------
                for b in range(bpc):
                    if b == 0:
                        for p in phase_a_pieces(0):
                            p()
                    if overlap_a and b + 1 < bpc:
                        lowq.extend(phase_a_pieces(b + 1))

                    for j in range(nqg):
                        for pi in range(2):
                            y_p = [y_pool.tile([P, qg], F32, name="y",
                                               tag="y") for _ in range(2)]
                            nch = cpq * (j + 1)
                            pend = None
                            for kc in range(nch):
                                r = kc - cpq * j
                                lo = max(r, 0) * P
                                ksl = slice(kc * P, (kc + 1) * P)
                                quad = quad_pool.tile([P, 2, qg], F32,
                                                      name="quad",
                                                      tag="quad")
                                for ci in range(2):
                                    h = 2 * pi + ci
                                    hp = slice(32 * h, 32 * h + 32)
                                    nc.tensor.matmul(
                                        quad[:, ci, lo:], kT[b][hp, ksl],
                                        qT[b][hp, j * qg + lo:(j + 1) * qg],
                                        start=True, stop=True,
                                        tile_position=(32 * h, 0))
                                drain(1)
                                et = expt.tile([P, 2, qg], F16, name="et",
                                               tag="et")
                                emit_exp(et[:, :, lo:], quad[:, :, lo:])
                                if r >= 0:
                                    tri_mask(et, lo)
                                if pend is not None:
                                    pe, plo, pkc = pend
                                    for ci in range(2):
                                        h = 2 * pi + ci
                                        nc.tensor.matmul(
                                            y_p[ci][:, plo:],
                                            vP[b][:, pkc, h, :],
                                            pe[:, ci, plo:],
                                            start=(pkc == 0), stop=False,
                                            skip_group_check=True)
                                pend = (et, lo, kc)
                            pe, plo, pkc = pend
                            for ci in range(2):
                                h = 2 * pi + ci
                                nc.tensor.matmul(
                                    y_p[ci][:, plo:], vP[b][:, pkc, h, :],
                                    pe[:, ci, plo:],
                                    start=(pkc == 0), stop=True,
                                    skip_group_check=True)
                            rec_box = [None]
                            tails.append(mk_c1(y_p, pi))
                            tails.append(mk_c2(pi, rec_box))
                            tails.append(mk_c3(y_p, pi, rec_box))
                        tails.append(mk_proj(b, j))

                while tails or lowq:
                    drain(1)
    nc.compile()
    return nc


_NC_CACHE = {}

CONFIG = {"exp_pat": "ADAA", "tri_engine": "pool",
          "quad_bufs": 2, "y_bufs": 4, "exp_bufs": 4, "overlap_a": True}


def _get_nc(bpc=BPC, t=T, loop_n=0):
    key = (bpc, t, loop_n)
    if key not in _NC_CACHE:
        _NC_CACHE[key] = build_attention_nc(bpc=bpc, t=t, loop_n=loop_n,
                                            **CONFIG)
    return _NC_CACHE[key]


def _run(x, w_attn, w_proj, **spmd_kwargs):
    x = np.ascontiguousarray(np.asarray(x), dtype=np.float32)
    w_attn = np.ascontiguousarray(np.asarray(w_attn), dtype=np.float32)
    w_proj = np.ascontiguousarray(np.asarray(w_proj), dtype=np.float32)
    assert x.shape == (B, T, D), x.shape

    nc = _get_nc()
    in_maps = [
        {"x": x[c * BPC:(c + 1) * BPC], "w_attn": w_attn, "w_proj": w_proj}
        for c in range(N_CORES)
    ]
    res = run_bass_kernel_spmd(nc, in_maps, list(range(N_CORES)),
                               **spmd_kwargs)
    out = np.concatenate([res.results[c]["out"] for c in range(N_CORES)],
                         axis=0)
    return out.astype(np.float32), res


def kernel(x, w_attn, w_proj):
    out, _ = _run(x, w_attn, w_proj)
    return out


if __name__ == "__main__":
    nc = build_attention_nc()
    print("built ok")
